# revision 2
# baseline (speedup 1.0000x reference)
"""Cross-attention Trainium2 kernel (Bass/Tile), 8-core SPMD.

Sharding: 8 cores = 2 (batch) x 4 (head groups of 3 heads).
Each core computes, for its (b, g):
    q^T = Wq_g @ x_b^T          [192, 2048]  (+bq)
    k^T = Wk_g @ y_b^T          [192, 2048]  (+bk)
    v   = y_b @ Wv_g^T          [2048, 192]
    per head: S^T = k_h q_h^T   [2048(m), 2048(l)] tiles in PSUM
              P^T = exp(S^T/8)  (softmax numerator, bf16)
              O^T = v_h^T P^T   (PSUM accumulated over m)
              den = 1^T P^T, O_n^T = O^T * (1/den)  (broadcast via PE)
    partial^T = Wp_g^T^T @ O_n^T  [768, 2048]  -> DRAM fp32
Host: out[b] = sum_g partial_g^T.T + Wp @ bv + bp.

All device matmuls in bf16 (fp32 PSUM accumulation); exp on ACT engine.
"""

import os
import sys
from contextlib import ExitStack

import numpy as np

for _p in ("/opt/trn_rl_repo", "/root/.axon_site/_ro/trn_rl_repo"):
    if os.path.isdir(_p) and _p not in sys.path:
        sys.path.insert(0, _p)

try:  # make trace=True work when the env lacks the NTFF hook module
    import antenv.axon_hooks  # noqa: F401
except Exception:
    import types

    _stub = types.ModuleType("antenv.axon_hooks")
    _stub._hook = None
    _stub.get_axon_ntff_profile_hook = lambda: _stub._hook

    def _set_hook(hook):
        _stub._hook = hook

    _stub.set_axon_ntff_profile_hook = _set_hook
    sys.modules["antenv.axon_hooks"] = _stub
    try:  # re-run the boot-time registration that failed without the module
        from trn_agent_boot.trn_boot import _ntff_profile_via_ctypes

        _stub._hook = _ntff_profile_via_ctypes("/opt/axon/libaxon_pjrt.so")
    except Exception:
        pass

import concourse.bass as bass
import concourse.tile as tile
from concourse import bacc as bacc_mod
from concourse import mybir
from concourse.bass_utils import run_bass_kernel_spmd
from ml_dtypes import bfloat16

F32 = mybir.dt.float32
F32R = mybir.dt.float32r
BF16 = mybir.dt.bfloat16
EXP = mybir.ActivationFunctionType.Exp

B = 2
L = 2048          # query length (also key length)
D = 768
HD = 64           # head dim
HPC = 3           # heads per core
GW = HPC * HD     # 192: head-group width
KT = D // 128     # 6 contraction tiles for V projection
KTA = KT + 1      # 7 tiles for Q/K: 7th carries the bias row (exact bias fold)
DA = KTA * 128    # 896: augmented contraction depth
NLC = L // 512    # 4 l-chunks
NM = L // 128     # 16 m-tiles
SCALE = 1.0 / 8.0  # hd ** -0.5


def _build_program(nkt_qk=KTA):
    nc = bacc_mod.Bacc()

    da = nkt_qk * 128
    xT = nc.dram_tensor("xT", [da, L], BF16, kind="ExternalInput")[:, :]
    yT = nc.dram_tensor("yT", [da, L], BF16, kind="ExternalInput")[:, :]
    wqT = nc.dram_tensor("wqT", [da, GW], BF16, kind="ExternalInput")[:, :]
    wkT = nc.dram_tensor("wkT", [da, GW], BF16, kind="ExternalInput")[:, :]
    wvT = nc.dram_tensor("wvT", [D, GW], BF16, kind="ExternalInput")[:, :]
    wpT = nc.dram_tensor("wpT", [GW, D], BF16, kind="ExternalInput")[:, :]
    pT = nc.dram_tensor("pT", [D, L], F32, kind="ExternalOutput")[:, :]

    with tile.TileContext(nc) as tc, ExitStack() as ctx:
        persist = ctx.enter_context(tc.tile_pool(name="persist", bufs=1))
        ppool = ctx.enter_context(tc.tile_pool(name="ppool", bufs=2, space="PSUM"))
        spool = ctx.enter_context(tc.tile_pool(name="spool", bufs=2, space="PSUM"))
        ptpool = ctx.enter_context(tc.tile_pool(name="ptpool", bufs=6))
        accpool = ctx.enter_context(tc.tile_pool(name="accpool", bufs=2))
        rpool = ctx.enter_context(tc.tile_pool(name="rpool", bufs=2))
        bcpool = ctx.enter_context(tc.tile_pool(name="bcpool", bufs=2))

        # ---------------- load inputs (chunked across DMA queues) ----------
        xT_sb = persist.tile([128, nkt_qk, L], BF16, tag="xT")
        yT_sb = persist.tile([128, nkt_qk, L], BF16, tag="yT")
        wq_sb = persist.tile([128, nkt_qk, GW], BF16, tag="wq")
        wk_sb = persist.tile([128, nkt_qk, GW], BF16, tag="wk")
        wv_sb = persist.tile([128, KT, GW], BF16, tag="wv")
        wp_a = persist.tile([128, D], BF16, tag="wpa")
        wp_b = persist.tile([64, D], BF16, tag="wpb")

        xT_r = xT.rearrange("(kt p) l -> p kt l", p=128)
        yT_r = yT.rearrange("(kt p) l -> p kt l", p=128)
        wk_r = wkT.rearrange("(kt p) g -> p kt g", p=128)
        wq_r = wqT.rearrange("(kt p) g -> p kt g", p=128)
        wv_r = wvT.rearrange("(kt p) g -> p kt g", p=128)
        # y l-half0 + weights + x l-half0 first: the first K/V/Q projection
        # chunks and slot-B attention only need those.
        h0 = slice(0, L // 2)
        h1 = slice(L // 2, L)
        for kt in range(nkt_qk):
            nc.sync.dma_start(out=yT_sb[:, kt, h0], in_=yT_r[:, kt, h0])
        nc.sync.dma_start(out=wk_sb, in_=wk_r)
        nc.sync.dma_start(out=wv_sb, in_=wv_r)
        nc.sync.dma_start(out=wq_sb, in_=wq_r)
        for kt in range(nkt_qk):
            nc.sync.dma_start(out=xT_sb[:, kt, h0], in_=xT_r[:, kt, h0])
        for kt in range(nkt_qk):
            nc.sync.dma_start(out=yT_sb[:, kt, h1], in_=yT_r[:, kt, h1])
        for kt in range(nkt_qk):
            nc.sync.dma_start(out=xT_sb[:, kt, h1], in_=xT_r[:, kt, h1])
        nc.sync.dma_start(out=wp_a, in_=wpT[0:128, :])
        nc.sync.dma_start(out=wp_b, in_=wpT[128:GW, :])

        ones_col = persist.tile([128, 1], BF16, tag="onesc")
        nc.vector.memset(ones_col, 1.0)

        # persistent activation tensors
        qT_p = persist.tile([128, L], BF16, tag="qTp")   # heads 0,1 stacked
        qT_2 = persist.tile([128, L], BF16, tag="qT2")   # head 2, dup halves
        kT_p = persist.tile([128, L], BF16, tag="kTp")
        kT_2 = persist.tile([128, L], BF16, tag="kT2")   # head 2, dup halves
        v_sb = persist.tile([128, NM, GW], BF16, tag="v")
        on_p = persist.tile([128, L], BF16, tag="onp")   # normalized O^T heads 0,1
        on_2 = persist.tile([64, L], BF16, tag="on2")    # head 2

        # ---------------- emission helpers ----------------
        def k_chunk(lc):
            sl = slice(lc * 512, (lc + 1) * 512)
            ps = ppool.tile([128, 512], F32, tag="ps")
            for kt in range(nkt_qk):
                nc.tensor.matmul(ps, wk_sb[:, kt, 0:128], yT_sb[:, kt, sl],
                                 start=(kt == 0), stop=(kt == nkt_qk - 1))
            nc.vector.tensor_copy(kT_p[:, sl], ps)
            ps2 = ppool.tile([64, 512], F32, tag="ps")
            for kt in range(nkt_qk):
                nc.tensor.matmul(ps2, wk_sb[:, kt, 128:GW], yT_sb[:, kt, sl],
                                 start=(kt == 0), stop=(kt == nkt_qk - 1))
            nc.vector.tensor_copy(kT_2[0:64, sl], ps2)
            nc.vector.tensor_copy(kT_2[64:128, sl], ps2)

        def v_chunk(m):
            ms = slice(m * 128, (m + 1) * 128)
            ps = ppool.tile([128, GW], F32, tag="ps")
            for kt in range(KT):
                nc.tensor.matmul(ps, yT_sb[:, kt, ms], wv_sb[:, kt, :],
                                 start=(kt == 0), stop=(kt == KT - 1))
            nc.vector.tensor_copy(v_sb[:, m, :], ps)

        def q_chunk(lc, pair):
            sl = slice(lc * 512, (lc + 1) * 512)
            if pair:
                ps = ppool.tile([128, 512], F32, tag="ps")
                for kt in range(nkt_qk):
                    nc.tensor.matmul(ps, wq_sb[:, kt, 0:128], xT_sb[:, kt, sl],
                                     start=(kt == 0), stop=(kt == nkt_qk - 1))
                nc.vector.tensor_copy(qT_p[:, sl], ps)
            else:
                ps2 = ppool.tile([64, 512], F32, tag="ps")
                for kt in range(nkt_qk):
                    nc.tensor.matmul(ps2, wq_sb[:, kt, 128:GW], xT_sb[:, kt, sl],
                                     start=(kt == 0), stop=(kt == nkt_qk - 1))
                nc.vector.tensor_copy(qT_2[0:64, sl], ps2)
                nc.vector.tensor_copy(qT_2[64:128, sl], ps2)

        def _acc_add(m, acc, pt):
            c = m % 2  # two interleaved chains halve the serial dep depth
            if m < 2:
                nc.vector.tensor_copy(acc[:, c, :], pt)
            else:
                nc.vector.tensor_add(acc[:, c, :], acc[:, c, :], pt)

        def merge_acc(acc):
            nc.vector.tensor_add(acc[:, 0, :], acc[:, 0, :], acc[:, 1, :])

        def norm_one(o_ap, acc, asl, dst_ap):
            """normalize one head/l-chunk: dst = o / den (acc chains merged)."""
            den = ppool.tile([1, 512], F32, tag="ps")
            nc.tensor.matmul(den, ones_col, acc[:, 0, asl], start=True, stop=True)
            recip = rpool.tile([1, 512], F32, tag="recip")
            nc.vector.reciprocal_approx_fast(out=recip, in_=den)
            bc = bcpool.tile([64, 512], F32, tag="bc")
            nc.gpsimd.partition_broadcast(bc, recip)
            nc.vector.tensor_mul(dst_ap, o_ap, bc)

        def slot_a_m(m, sl, s_ps, o_ps, acc, first, last):
            ms = slice(m * 128, (m + 1) * 128)
            nc.tensor.matmul(s_ps[:, 0:512], kT_p[0:64, ms], qT_p[0:64, sl],
                             tile_position=(0, 0), start=True, stop=True)
            nc.tensor.matmul(s_ps[:, 512:1024], kT_p[64:128, ms], qT_p[64:128, sl],
                             tile_position=(64, 0), start=True, stop=True)
            pt = ptpool.tile([128, 1024], BF16, tag="pt")
            nc.scalar.activation(pt, s_ps, EXP, scale=SCALE)
            nc.tensor.matmul(o_ps[0:64, :], v_sb[:, m, 0:64], pt[:, 0:512],
                             tile_position=(0, 0), start=first, stop=last)
            nc.tensor.matmul(o_ps[64:128, :], v_sb[:, m, 64:128], pt[:, 512:1024],
                             tile_position=(0, 64), start=first, stop=last)
            _acc_add(m, acc, pt)

        def slot_b_m(m, sl0, sl1, s_ps, o_ps, acc, first, last):
            ms = slice(m * 128, (m + 1) * 128)
            nc.tensor.matmul(s_ps[:, 0:512], kT_2[0:64, ms], qT_2[0:64, sl0],
                             tile_position=(0, 0), start=True, stop=True)
            nc.tensor.matmul(s_ps[:, 512:1024], kT_2[64:128, ms], qT_2[64:128, sl1],
                             tile_position=(64, 0), start=True, stop=True)
            pt = ptpool.tile([128, 1024], BF16, tag="pt")
            nc.scalar.activation(pt, s_ps, EXP, scale=SCALE)
            nc.tensor.matmul(o_ps[0:64, :], v_sb[:, m, 128:GW], pt[:, 0:512],
                             tile_position=(0, 0), start=first, stop=last)
            nc.tensor.matmul(o_ps[64:128, :], v_sb[:, m, 128:GW], pt[:, 512:1024],
                             tile_position=(0, 64), start=first, stop=last)
            _acc_add(m, acc, pt)

        def p_proj(lc):
            sl = slice(lc * 512, (lc + 1) * 512)
            for o in range(D // 128):
                osl = slice(o * 128, (o + 1) * 128)
                ps = ppool.tile([128, 512], F32, tag="ps")
                nc.tensor.matmul(ps, wp_a[:, osl], on_p[:, sl], start=True, stop=False)
                nc.tensor.matmul(ps, wp_b[:, osl], on_2[:, sl], start=False, stop=True)
                po = ptpool.tile([128, 512], F32, tag="po")
                nc.vector.tensor_copy(po, ps)
                nc.sync.dma_start(out=pT[osl, sl], in_=po)

        # ---------------- software-pipelined emission ----------------
        # head-2 slot first, with K/V projection chunks threaded between its
        # m-blocks so ACT starts ~10us in and PE never idles on a phase edge.
        k_chunk(0)
        for m in range(4):
            v_chunk(m)
        q_chunk(0, pair=False)
        q_chunk(1, pair=False)

        # slot B, lc-pair 0 (covers l chunks 0,1)
        o_ps_b0 = ppool.tile([128, 512], F32, tag="ps")
        acc_b0 = accpool.tile([128, 2, 1024], BF16, tag="acc")
        sl0, sl1 = slice(0, 512), slice(512, 1024)
        for m in range(NM):
            s_ps = spool.tile([128, 1024], F32, tag="s")
            slot_b_m(m, sl0, sl1, s_ps, o_ps_b0, acc_b0, m == 0, m == NM - 1)
            if m == 3:
                k_chunk(1)
                for mm in range(4, 8):
                    v_chunk(mm)
            elif m == 7:
                k_chunk(2)
                for mm in range(8, 12):
                    v_chunk(mm)
            elif m == 11:
                k_chunk(3)
                for mm in range(12, 16):
                    v_chunk(mm)
        merge_acc(acc_b0)
        norm_one(o_ps_b0[0:64, :], acc_b0, slice(0, 512), on_2[:, sl0])
        norm_one(o_ps_b0[64:128, :], acc_b0, slice(512, 1024), on_2[:, sl1])
        q_chunk(2, pair=False)
        q_chunk(3, pair=False)

        # slot B, lc-pair 1 (l chunks 2,3)
        o_ps_b1 = ppool.tile([128, 512], F32, tag="ps")
        acc_b1 = accpool.tile([128, 2, 1024], BF16, tag="acc")
        sl2, sl3 = slice(1024, 1536), slice(1536, 2048)
        for m in range(NM):
            s_ps = spool.tile([128, 1024], F32, tag="s")
            slot_b_m(m, sl2, sl3, s_ps, o_ps_b1, acc_b1, m == 0, m == NM - 1)
            if m == 3:
                q_chunk(0, pair=True)
            elif m == 7:
                q_chunk(1, pair=True)
            elif m == 11:
                q_chunk(2, pair=True)
        merge_acc(acc_b1)
        norm_one(o_ps_b1[0:64, :], acc_b1, slice(0, 512), on_2[:, sl2])
        norm_one(o_ps_b1[64:128, :], acc_b1, slice(512, 1024), on_2[:, sl3])

        # slot A per l-chunk, with q-proj of the next chunk and p-proj of the
        # previous chunk threaded into the m-loop
        for lc in range(NLC):
            sl = slice(lc * 512, (lc + 1) * 512)
            o_ps = ppool.tile([128, 512], F32, tag="ps")
            acc = accpool.tile([128, 2, 1024], BF16, tag="acc")
            for m in range(NM):
                s_ps = spool.tile([128, 1024], F32, tag="s")
                slot_a_m(m, sl, s_ps, o_ps, acc, m == 0, m == NM - 1)
                if m == 7 and lc == 0:
                    q_chunk(3, pair=True)
                elif m == 7 and lc > 0:
                    p_proj(lc - 1)
            merge_acc(acc)
            norm_one(o_ps[0:64, :], acc, slice(0, 512), on_p[0:64, sl])
            norm_one(o_ps[64:128, :], acc, slice(512, 1024), on_p[64:128, sl])
        p_proj(NLC - 1)

    nc.finalize()
    return nc




def _aug_act(a, aug):
    """[L, D] activations -> [da, L]: transpose (+ ones row + zero pad)."""
    if not aug:
        return np.ascontiguousarray(a.T).astype(bfloat16)
    out = np.zeros((DA, L), dtype=bfloat16)
    out[:D] = a.T.astype(bfloat16)
    out[D] = 1.0
    return out


def _aug_w(w_rows, b_rows, aug):
    """[GW, D] weight rows (+ [GW] bias) -> [da, GW] lhsT."""
    if not aug:
        return np.ascontiguousarray(w_rows.T).astype(bfloat16)
    out = np.zeros((DA, GW), dtype=bfloat16)
    out[:D] = w_rows.T.astype(bfloat16)
    out[D] = b_rows.astype(bfloat16)
    return out


def _make_in_maps(x, y, Wq, bq, Wk, bk, Wv, bv, Wp, bp, aug):
    in_maps = []
    xTs = [_aug_act(x[b], aug) for b in range(B)]
    yTs = [_aug_act(y[b], aug) for b in range(B)]
    for core in range(8):
        b, g = divmod(core, 4)
        rows = slice(g * GW, (g + 1) * GW)
        in_maps.append({
            "xT": xTs[b],
            "yT": yTs[b],
            "wqT": _aug_w(Wq[rows], bq[rows], aug),
            "wkT": _aug_w(Wk[rows], bk[rows], aug),
            "wvT": np.ascontiguousarray(Wv[rows].T).astype(bfloat16),
            "wpT": np.ascontiguousarray(Wp[:, rows].T).astype(bfloat16),
        })
    return in_maps


def _combine(results, Wv, Wp, bp, bv):
    out = np.zeros((B, L, D), dtype=np.float32)
    for core in range(8):
        b = core // 4
        out[b] += results[core]["pT"].T
    out += (Wp @ bv + bp)[None, None, :]
    return out


_NC = {}


def _get_nc(aug=True):
    if aug not in _NC:
        _NC[aug] = _build_program(KTA if aug else KT)
    return _NC[aug]


def run(inputs, trace=False, trace_cores=None, **kwargs):
    aug = bool(np.any(inputs["bq"]) or np.any(inputs["bk"]))
    nc = _get_nc(aug)
    in_maps = _make_in_maps(aug=aug, **inputs)
    res = run_bass_kernel_spmd(
        nc, in_maps, core_ids=list(range(8)), trace=trace,
        trace_cores=trace_cores, **kwargs)
    out = _combine(res.results, inputs["Wv"], inputs["Wp"],
                   inputs["bp"], inputs["bv"])
    return out, res


def kernel(**inputs):
    inputs = {k: np.asarray(v) for k, v in inputs.items()}
    out, _ = run(inputs, trace=False)
    return out



# revision 8
# speedup vs baseline: 1.0728x; 1.0728x over previous
"""Cross-attention Trainium2 kernel (Bass/Tile), 8-core SPMD.

Sharding: 8 cores = 2 (batch) x 4 (head groups of 3 heads).
Each core computes, for its (b, g):
    q^T = Wq_g @ x_b^T          [192, 2048]  (+bq)
    k^T = Wk_g @ y_b^T          [192, 2048]  (+bk)
    v   = y_b @ Wv_g^T          [2048, 192]
    per head: S^T = k_h q_h^T   [2048(m), 2048(l)] tiles in PSUM
              P^T = exp(S^T/8)  (softmax numerator, bf16)
              O^T = v_h^T P^T   (PSUM accumulated over m)
              den = 1^T P^T (DVE-accumulated), O_n^T = O^T * (1/den)
    partial^T = Wp_g^T^T @ O_n^T  [768, 2048]  -> DRAM fp32
Host: out[b] = sum_g partial_g^T.T + Wp @ bv + bp.

The schedule is exp-driven: the ScalarE (ACT) runs one [128,1024] exp per
m-iteration back-to-back (~1.05us each, 96 total); all projection work is
split into <=0.7us PE units threaded one-or-two per iteration so the PE
never starves the ACT pipe. Head-2 tensors are produced pre-duplicated
across both partition halves by duplicated weight columns (no DVE copies).
"""

import os
import sys
from contextlib import ExitStack

import numpy as np

for _p in ("/opt/trn_rl_repo", "/root/.axon_site/_ro/trn_rl_repo"):
    if os.path.isdir(_p) and _p not in sys.path:
        sys.path.insert(0, _p)

try:  # make trace=True work when the env lacks the NTFF hook module
    import antenv.axon_hooks  # noqa: F401
except Exception:
    import types

    _stub = types.ModuleType("antenv.axon_hooks")
    _stub._hook = None
    _stub.get_axon_ntff_profile_hook = lambda: _stub._hook

    def _set_hook(hook):
        _stub._hook = hook

    _stub.set_axon_ntff_profile_hook = _set_hook
    sys.modules["antenv.axon_hooks"] = _stub
    try:  # re-run the boot-time registration that failed without the module
        from trn_agent_boot.trn_boot import _ntff_profile_via_ctypes

        _stub._hook = _ntff_profile_via_ctypes("/opt/axon/libaxon_pjrt.so")
    except Exception:
        pass

import concourse.bass as bass
import concourse.tile as tile
from concourse import bacc as bacc_mod
from concourse import mybir
from concourse.bass_utils import run_bass_kernel_spmd
from ml_dtypes import bfloat16

F32 = mybir.dt.float32
BF16 = mybir.dt.bfloat16
EXP = mybir.ActivationFunctionType.Exp

B = 2
L = 2048          # query length (also key length)
D = 768
HD = 64           # head dim
GW = 192          # head-group width (3 heads per core)
KT = D // 128     # 6 contraction tiles
KTA = KT + 1      # 7 tiles when a bias row is needed
NM = L // 128     # 16 m-tiles
SCALE = 1.0 / 8.0  # hd ** -0.5


def _build_program(nkt=KT):
    nc = bacc_mod.Bacc()

    xT = nc.dram_tensor("xT", [128, nkt * L], BF16, kind="ExternalInput")[:, :]
    yT = nc.dram_tensor("yT", [128, nkt * L], BF16, kind="ExternalInput")[:, :]
    wq = nc.dram_tensor("wq", [128, nkt * 256], BF16, kind="ExternalInput")[:, :]
    wk = nc.dram_tensor("wk", [128, nkt * 256], BF16, kind="ExternalInput")[:, :]
    wv = nc.dram_tensor("wv", [128, KT * GW], BF16, kind="ExternalInput")[:, :]
    wpa = nc.dram_tensor("wpa", [128, D], BF16, kind="ExternalInput")[:, :]
    wpb = nc.dram_tensor("wpb", [64, D], BF16, kind="ExternalInput")[:, :]
    pT = nc.dram_tensor("pT", [D, L], F32, kind="ExternalOutput")[:, :]

    xT_r = xT.rearrange("p (kt l) -> p kt l", kt=nkt)
    yT_r = yT.rearrange("p (kt l) -> p kt l", kt=nkt)
    wq_r = wq.rearrange("p (kt g) -> p kt g", kt=nkt)
    wk_r = wk.rearrange("p (kt g) -> p kt g", kt=nkt)
    wv_r = wv.rearrange("p (kt g) -> p kt g", kt=KT)

    with tile.TileContext(nc) as tc, ExitStack() as ctx:
        persist = ctx.enter_context(tc.tile_pool(name="persist", bufs=1))
        spool = ctx.enter_context(tc.tile_pool(name="spool", bufs=2, space="PSUM"))
        opool = ctx.enter_context(tc.tile_pool(name="opool", bufs=2, space="PSUM"))
        ppool = ctx.enter_context(tc.tile_pool(name="ppool", bufs=2, space="PSUM"))
        ptpool = ctx.enter_context(tc.tile_pool(name="ptpool", bufs=4))
        accpool = ctx.enter_context(tc.tile_pool(name="accpool", bufs=2))
        rpool = ctx.enter_context(tc.tile_pool(name="rpool", bufs=2))
        bcpool = ctx.enter_context(tc.tile_pool(name="bcpool", bufs=2))
        popool = ctx.enter_context(tc.tile_pool(name="popool", bufs=2))

        # ---------------- persistent SBUF ----------------
        xT_sb = persist.tile([128, nkt, L], BF16, tag="xT")
        yT_sb = persist.tile([128, nkt, L], BF16, tag="yT")
        wq_sb = persist.tile([128, nkt, 256], BF16, tag="wq")
        wk_sb = persist.tile([128, nkt, 256], BF16, tag="wk")
        wv_sb = persist.tile([128, KT, GW], BF16, tag="wv")
        wpa_sb = persist.tile([128, D], BF16, tag="wpa")
        wpb_sb = persist.tile([64, D], BF16, tag="wpb")

        qT_p = persist.tile([128, L], BF16, tag="qTp")   # heads 0,1 stacked
        qT_2 = persist.tile([128, L], BF16, tag="qT2")   # head 2, dup halves
        kT_p = persist.tile([128, L], BF16, tag="kTp")
        kT_2 = persist.tile([128, L], BF16, tag="kT2")
        v_sb = persist.tile([128, NM, GW], BF16, tag="v")
        on_p = persist.tile([128, L], BF16, tag="onp")   # normalized O^T heads 0,1
        on_2 = persist.tile([64, L], BF16, tag="on2")    # head 2

        ones_col = persist.tile([128, 1], BF16, tag="onesc")
        scr = persist.tile([128, 64], BF16, tag="scr")
        nc.vector.memset(ones_col, 1.0)
        nc.vector.memset(scr, 1.0)

        # ---------------- input DMA, need-ordered ----------------
        # gates: k0h2 <- wk + yT[:512]; q2(0,1) <- wq + xT[:1024];
        # v(0..7) <- wv + yT[:1024]; later halves stream in behind.
        h0, h1 = slice(0, L // 2), slice(L // 2, L)
        nc.sync.dma_start(out=wk_sb, in_=wk_r)
        for kt in range(nkt):
            nc.sync.dma_start(out=yT_sb[:, kt, 0:512], in_=yT_r[:, kt, 0:512])
        nc.sync.dma_start(out=wq_sb, in_=wq_r)
        for kt in range(nkt):
            nc.sync.dma_start(out=xT_sb[:, kt, h0], in_=xT_r[:, kt, h0])
        nc.sync.dma_start(out=wv_sb, in_=wv_r)
        for kt in range(nkt):
            nc.sync.dma_start(out=yT_sb[:, kt, 512:1024], in_=yT_r[:, kt, 512:1024])
        for kt in range(nkt):
            nc.sync.dma_start(out=yT_sb[:, kt, h1], in_=yT_r[:, kt, h1])
        for kt in range(nkt):
            nc.sync.dma_start(out=xT_sb[:, kt, h1], in_=xT_r[:, kt, h1])
        nc.sync.dma_start(out=wpa_sb, in_=wpa)
        nc.sync.dma_start(out=wpb_sb, in_=wpb)

        def aux_ps():
            return ppool.tile([128, 512], F32, tag="aux", name="aux_ps")

        # ---------------- PE warm-up during the DMA wait ----------------
        wps = aux_ps()
        for _ in range(40):
            nc.tensor.matmul(wps[0:64, 0:64], scr, scr, start=True, stop=True)

        # ---------------- unit emitters ----------------
        def make_kq(j, half, w_sb, act_sb, dst):
            """K/Q projection for l/m-chunk j: two ~0.65us parts."""
            sl = slice(j * 512, (j + 1) * 512)
            cs = slice(128 * half, 128 * (half + 1))
            st = {}

            def part(p):
                if p == 0:
                    st["ps"] = aux_ps()
                ps = st["ps"]
                k0, k1 = (0, nkt // 2) if p == 0 else (nkt // 2, nkt)
                for kt in range(k0, k1):
                    nc.tensor.matmul(ps, w_sb[:, kt, cs], act_sb[:, kt, sl],
                                     start=(kt == 0), stop=(kt == nkt - 1))
                if p == 1:
                    nc.vector.tensor_copy(dst[:, sl], ps)
            return part

        def v_unit(m):
            ms = slice(m * 128, (m + 1) * 128)
            ps = aux_ps()
            for kt in range(KT):
                nc.tensor.matmul(ps[:, 0:GW], yT_sb[:, kt, ms], wv_sb[:, kt, :],
                                 start=(kt == 0), stop=(kt == KT - 1))
            nc.vector.tensor_copy(v_sb[:, m, :], ps[:, 0:GW])

        def p_unit(lc, o):
            sl = slice(lc * 512, (lc + 1) * 512)
            osl = slice(o * 128, (o + 1) * 128)
            ps = aux_ps()
            nc.tensor.matmul(ps, wpa_sb[:, osl], on_p[:, sl], start=True, stop=False)
            nc.tensor.matmul(ps, wpb_sb[:, osl], on_2[:, sl], start=False, stop=True)
            po = popool.tile([128, 512], F32, tag="po")
            nc.vector.tensor_copy(po, ps)
            nc.sync.dma_start(out=pT[osl, sl], in_=po)

        # ---------------- attention iterations ----------------
        def iter_a(lc, m, o_ps, acc):
            sl = slice(lc * 512, (lc + 1) * 512)
            ms = slice(m * 128, (m + 1) * 128)
            s_ps = spool.tile([128, 1024], F32, tag="s")
            nc.tensor.matmul(s_ps[:, 0:512], kT_p[0:64, ms], qT_p[0:64, sl],
                             tile_position=(0, 0), start=True, stop=True)
            nc.tensor.matmul(s_ps[:, 512:1024], kT_p[64:128, ms], qT_p[64:128, sl],
                             tile_position=(64, 0), start=True, stop=True)
            pt = acc if m == 0 else ptpool.tile([128, 1024], BF16, tag="pt")
            nc.scalar.activation(pt, s_ps, EXP, scale=SCALE)
            nc.tensor.matmul(o_ps[0:64, :], v_sb[:, m, 0:64], pt[:, 0:512],
                             tile_position=(0, 0), start=(m == 0), stop=(m == NM - 1),
                             skip_group_check=True)
            nc.tensor.matmul(o_ps[64:128, :], v_sb[:, m, 64:128], pt[:, 512:1024],
                             tile_position=(0, 64), start=(m == 0), stop=(m == NM - 1),
                             skip_group_check=True)
            if m > 0:
                nc.vector.tensor_add(acc, acc, pt)

        def iter_b(pair, m, o_ps, acc):
            sl0 = slice(pair * 1024, pair * 1024 + 512)
            sl1 = slice(pair * 1024 + 512, (pair + 1) * 1024)
            ms = slice(m * 128, (m + 1) * 128)
            s_ps = spool.tile([128, 1024], F32, tag="s")
            nc.tensor.matmul(s_ps[:, 0:512], kT_2[0:64, ms], qT_2[0:64, sl0],
                             tile_position=(0, 0), start=True, stop=True)
            nc.tensor.matmul(s_ps[:, 512:1024], kT_2[64:128, ms], qT_2[64:128, sl1],
                             tile_position=(64, 0), start=True, stop=True)
            pt = acc if m == 0 else ptpool.tile([128, 1024], BF16, tag="pt")
            nc.scalar.activation(pt, s_ps, EXP, scale=SCALE)
            nc.tensor.matmul(o_ps[0:64, :], v_sb[:, m, 128:GW], pt[:, 0:512],
                             tile_position=(0, 0), start=(m == 0), stop=(m == NM - 1),
                             skip_group_check=True)
            nc.tensor.matmul(o_ps[64:128, :], v_sb[:, m, 128:GW], pt[:, 512:1024],
                             tile_position=(0, 64), start=(m == 0), stop=(m == NM - 1),
                             skip_group_check=True)
            if m > 0:
                nc.vector.tensor_add(acc, acc, pt)

        def run_group(kind, idx, extras):
            o_ps = opool.tile([128, 512], F32, tag="o")
            acc = accpool.tile([128, 1024], BF16, tag="acc")
            for m in range(NM):
                for fn in extras.get(m, ()):
                    fn()
                if kind == "A":
                    iter_a(idx, m, o_ps, acc)
                else:
                    iter_b(idx, m, o_ps, acc)
            # softmax denominators + normalization
            den0 = aux_ps()
            nc.tensor.matmul(den0[0:1, :], ones_col, acc[:, 0:512], start=True, stop=True)
            den1 = aux_ps()
            nc.tensor.matmul(den1[0:1, :], ones_col, acc[:, 512:1024], start=True, stop=True)
            r0 = rpool.tile([1, 512], F32, tag="r")
            nc.vector.reciprocal_approx_fast(out=r0, in_=den0[0:1, :])
            r1 = rpool.tile([1, 512], F32, tag="r")
            nc.vector.reciprocal_approx_fast(out=r1, in_=den1[0:1, :])
            bc0 = bcpool.tile([64, 512], F32, tag="bc0")
            bc1 = bcpool.tile([64, 512], F32, tag="bc1")
            nc.gpsimd.partition_broadcast(bc0, r0)
            nc.gpsimd.partition_broadcast(bc1, r1)
            if kind == "A":
                sl = slice(idx * 512, (idx + 1) * 512)
                nc.vector.tensor_mul(on_p[0:64, sl], o_ps[0:64, :], bc0)
                nc.vector.tensor_mul(on_p[64:128, sl], o_ps[64:128, :], bc1)
            else:
                sl0 = slice(idx * 1024, idx * 1024 + 512)
                sl1 = slice(idx * 1024 + 512, (idx + 1) * 1024)
                nc.vector.tensor_mul(on_2[:, sl0], o_ps[0:64, :], bc0)
                nc.vector.tensor_mul(on_2[:, sl1], o_ps[64:128, :], bc1)

        # ---------------- prologue ----------------
        k0h2 = make_kq(0, 1, wk_sb, yT_sb, kT_2)
        k0h2(0); k0h2(1)
        q2_0 = make_kq(0, 1, wq_sb, xT_sb, qT_2)
        q2_0(0); q2_0(1)
        q2_1 = make_kq(1, 1, wq_sb, xT_sb, qT_2)
        q2_1(0); q2_1(1)
        v_unit(0)
        v_unit(1)

        # ---------------- group schedule ----------------
        # B0 covers head2 x l[0:1024]; extras: v2..15, k1..3-h2, k0-h01
        k1h2 = make_kq(1, 1, wk_sb, yT_sb, kT_2)
        k2h2 = make_kq(2, 1, wk_sb, yT_sb, kT_2)
        k3h2 = make_kq(3, 1, wk_sb, yT_sb, kT_2)
        k0h01 = make_kq(0, 0, wk_sb, yT_sb, kT_p)
        run_group("B", 0, {
            0: [lambda: v_unit(2)],
            1: [lambda: v_unit(3)],
            2: [lambda: k1h2(0)],
            3: [lambda: k1h2(1), lambda: v_unit(4)],
            4: [lambda: v_unit(5)],
            5: [lambda: v_unit(6)],
            6: [lambda: k2h2(0), lambda: v_unit(7)],
            7: [lambda: k2h2(1), lambda: v_unit(8)],
            8: [lambda: v_unit(9)],
            9: [lambda: v_unit(10)],
            10: [lambda: k3h2(0), lambda: v_unit(11)],
            11: [lambda: k3h2(1), lambda: v_unit(12)],
            12: [lambda: v_unit(13)],
            13: [lambda: v_unit(14)],
            14: [lambda: v_unit(15), lambda: k0h01(0)],
            15: [lambda: k0h01(1)],
        })

        # A0 covers heads 0,1 x l[0:512]; extras: k1..3-h01, q0-h01 (!), q2_2, q2_3
        q0h01 = make_kq(0, 0, wq_sb, xT_sb, qT_p)
        k1h01 = make_kq(1, 0, wk_sb, yT_sb, kT_p)
        k2h01 = make_kq(2, 0, wk_sb, yT_sb, kT_p)
        k3h01 = make_kq(3, 0, wk_sb, yT_sb, kT_p)
        q2_2 = make_kq(2, 1, wq_sb, xT_sb, qT_2)
        q2_3 = make_kq(3, 1, wq_sb, xT_sb, qT_2)
        # q0h01 must precede A0's first S: emit in B0's tail shadow
        q0h01(0); q0h01(1)
        run_group("A", 0, {
            0: [lambda: k1h01(0)],
            1: [lambda: k1h01(1)],
            3: [lambda: q2_2(0)],
            4: [lambda: q2_2(1)],
            6: [lambda: k2h01(0)],
            7: [lambda: k2h01(1)],
            8: [lambda: q2_3(0)],
            9: [lambda: q2_3(1)],
            10: [lambda: k3h01(0)],
            11: [lambda: k3h01(1)],
        })

        # B1 covers head2 x l[1024:2048]; extras: p_proj(0), q1-h01
        q1h01 = make_kq(1, 0, wq_sb, xT_sb, qT_p)
        run_group("B", 1, {
            1: [lambda: p_unit(0, 0)],
            3: [lambda: p_unit(0, 1)],
            5: [lambda: p_unit(0, 2)],
            7: [lambda: p_unit(0, 3)],
            9: [lambda: p_unit(0, 4)],
            11: [lambda: p_unit(0, 5)],
            13: [lambda: q1h01(0)],
            14: [lambda: q1h01(1)],
        })

        # A1: extras: q2-h01
        q2h01 = make_kq(2, 0, wq_sb, xT_sb, qT_p)
        run_group("A", 1, {
            5: [lambda: q2h01(0)],
            6: [lambda: q2h01(1)],
        })

        # A2: extras: p_proj(1), q3-h01
        q3h01 = make_kq(3, 0, wq_sb, xT_sb, qT_p)
        run_group("A", 2, {
            1: [lambda: p_unit(1, 0)],
            3: [lambda: p_unit(1, 1)],
            5: [lambda: p_unit(1, 2)],
            7: [lambda: p_unit(1, 3)],
            9: [lambda: p_unit(1, 4)],
            11: [lambda: p_unit(1, 5)],
            13: [lambda: q3h01(0)],
            14: [lambda: q3h01(1)],
        })

        # A3: extras: p_proj(2)
        run_group("A", 3, {
            1: [lambda: p_unit(2, 0)],
            3: [lambda: p_unit(2, 1)],
            5: [lambda: p_unit(2, 2)],
            7: [lambda: p_unit(2, 3)],
            9: [lambda: p_unit(2, 4)],
            11: [lambda: p_unit(2, 5)],
        })

        # tail: p_proj(3)
        for o in range(6):
            p_unit(3, o)

    nc.finalize()
    return nc


def _pack_kt(t):
    """[da, W] -> [128, nkt*W] (kt-major blocks interleaved to partitions)."""
    da, w = t.shape
    nkt = da // 128
    return np.ascontiguousarray(
        t.reshape(nkt, 128, w).transpose(1, 0, 2).reshape(128, nkt * w)
    ).astype(bfloat16)


def _prep_act(a, aug):
    """[L, D] activations -> [128, nkt*L] (+ ones row when aug)."""
    da = (KTA if aug else KT) * 128
    t = np.zeros((da, L), dtype=np.float32)
    t[:D] = a.T
    if aug:
        t[D] = 1.0
    return _pack_kt(t)


def _prep_wqk(w_rows, b_rows, aug):
    """[GW, D] weight rows (+ [GW] bias) -> [128, nkt*256] packed lhsT.

    cols 0:128 = heads 0,1; cols 128:192 = head 2; cols 192:256 = head 2 dup.
    """
    da = (KTA if aug else KT) * 128
    t = np.zeros((da, 256), dtype=np.float32)
    t[:D, 0:128] = w_rows[0:128].T
    t[:D, 128:192] = w_rows[128:192].T
    t[:D, 192:256] = w_rows[128:192].T
    if aug:
        t[D, 0:128] = b_rows[0:128]
        t[D, 128:192] = b_rows[128:192]
        t[D, 192:256] = b_rows[128:192]
    return _pack_kt(t)


def _make_in_maps(x, y, Wq, bq, Wk, bk, Wv, bv, Wp, bp, aug):
    in_maps = []
    xTs = [_prep_act(np.asarray(x[b]), aug) for b in range(B)]
    yTs = [_prep_act(np.asarray(y[b]), aug) for b in range(B)]
    for core in range(8):
        b, g = divmod(core, 4)
        rows = slice(g * GW, (g + 1) * GW)
        wvT = np.zeros((KT * 128, GW), dtype=np.float32)
        wvT[:D] = Wv[rows].T
        in_maps.append({
            "xT": xTs[b],
            "yT": yTs[b],
            "wq": _prep_wqk(Wq[rows], bq[rows], aug),
            "wk": _prep_wqk(Wk[rows], bk[rows], aug),
            "wv": _pack_kt(wvT),
            "wpa": np.ascontiguousarray(Wp[:, rows].T[0:128]).astype(bfloat16),
            "wpb": np.ascontiguousarray(Wp[:, rows].T[128:GW]).astype(bfloat16),
        })
    return in_maps


def _combine(results, Wv, Wp, bp, bv):
    out = np.zeros((B, L, D), dtype=np.float32)
    for core in range(8):
        b = core // 4
        out[b] += results[core]["pT"].T
    out += (Wp @ bv + bp)[None, None, :]
    return out


_NC = {}


def _get_nc(aug):
    if aug not in _NC:
        _NC[aug] = _build_program(KTA if aug else KT)
    return _NC[aug]


def run(inputs, trace=False, trace_cores=None, **kwargs):
    aug = bool(np.any(inputs["bq"]) or np.any(inputs["bk"]))
    nc = _get_nc(aug)
    in_maps = _make_in_maps(aug=aug, **inputs)
    res = run_bass_kernel_spmd(
        nc, in_maps, core_ids=list(range(8)), trace=trace,
        trace_cores=trace_cores, **kwargs)
    out = _combine(res.results, inputs["Wv"], inputs["Wp"],
                   inputs["bp"], inputs["bv"])
    return out, res


def kernel(**inputs):
    inputs = {k: np.asarray(v) for k, v in inputs.items()}
    out, _ = run(inputs, trace=False)
    return out


# revision 15
# speedup vs baseline: 1.0923x; 1.0181x over previous
"""Cross-attention Trainium2 kernel (Bass/Tile), 8-core SPMD.

Sharding: 8 cores = 2 (batch) x 4 (head groups of 3 heads).
Each core computes, for its (b, g):
    q^T = Wq_g @ x_b^T          [192, 2048]  (+bq)
    k^T = Wk_g @ y_b^T          [192, 2048]  (+bk)
    v   = y_b @ Wv_g^T          [2048, 192]
    per head: S^T = k_h q_h^T   [2048(m), 2048(l)] tiles in PSUM
              P^T = exp(S^T/8)  (softmax numerator, bf16)
              O^T = v_h^T P^T   (PSUM accumulated over m)
              den = 1^T P^T (DVE-accumulated), O_n^T = O^T * (1/den)
    partial^T = Wp_g^T^T @ O_n^T  [768, 2048]  -> DRAM fp32
Host: out[b] = sum_g partial_g^T.T + Wp @ bv + bp.

The schedule is exp-driven: the ScalarE (ACT) runs one [128,1024] exp per
m-iteration back-to-back (~1.05us each, 96 total); all projection work is
split into <=0.7us PE units threaded one-or-two per iteration so the PE
never starves the ACT pipe. Head-2 tensors are produced pre-duplicated
across both partition halves by duplicated weight columns (no DVE copies).
"""

import os
import sys
from contextlib import ExitStack

import numpy as np

for _p in ("/opt/trn_rl_repo", "/root/.axon_site/_ro/trn_rl_repo"):
    if os.path.isdir(_p) and _p not in sys.path:
        sys.path.insert(0, _p)

try:  # make trace=True work when the env lacks the NTFF hook module
    import antenv.axon_hooks  # noqa: F401
except Exception:
    import types

    _stub = types.ModuleType("antenv.axon_hooks")
    _stub._hook = None
    _stub.get_axon_ntff_profile_hook = lambda: _stub._hook

    def _set_hook(hook):
        _stub._hook = hook

    _stub.set_axon_ntff_profile_hook = _set_hook
    sys.modules["antenv.axon_hooks"] = _stub
    try:  # re-run the boot-time registration that failed without the module
        from trn_agent_boot.trn_boot import _ntff_profile_via_ctypes

        _stub._hook = _ntff_profile_via_ctypes("/opt/axon/libaxon_pjrt.so")
    except Exception:
        pass

import concourse.bass as bass
import concourse.tile as tile
from concourse import bacc as bacc_mod
from concourse import mybir
from concourse.bass_utils import run_bass_kernel_spmd
from ml_dtypes import bfloat16

F32 = mybir.dt.float32
BF16 = mybir.dt.bfloat16
EXP = mybir.ActivationFunctionType.Exp

B = 2
L = 2048          # query length (also key length)
D = 768
HD = 64           # head dim
GW = 192          # head-group width (3 heads per core)
KT = D // 128     # 6 contraction tiles
KTA = KT + 1      # 7 tiles when a bias row is needed
NM = L // 128     # 16 m-tiles
SCALE = 1.0 / 8.0  # hd ** -0.5

WARM_TAIL = False
WARM_GROUP = False
SCALAR_PO = False


def _build_program(nkt=KT):
    nc = bacc_mod.Bacc()

    xT = nc.dram_tensor("xT", [128, nkt * L], BF16, kind="ExternalInput")[:, :]
    yT = nc.dram_tensor("yT", [128, nkt * L], BF16, kind="ExternalInput")[:, :]
    wq = nc.dram_tensor("wq", [128, nkt * 256], BF16, kind="ExternalInput")[:, :]
    wk = nc.dram_tensor("wk", [128, nkt * 256], BF16, kind="ExternalInput")[:, :]
    wv = nc.dram_tensor("wv", [128, KT * GW], BF16, kind="ExternalInput")[:, :]
    wpa = nc.dram_tensor("wpa", [128, D], BF16, kind="ExternalInput")[:, :]
    wpb = nc.dram_tensor("wpb", [64, D], BF16, kind="ExternalInput")[:, :]
    pT = nc.dram_tensor("pT", [D, L], F32, kind="ExternalOutput")[:, :]

    xT_r = xT.rearrange("p (kt l) -> p kt l", kt=nkt)
    yT_r = yT.rearrange("p (kt l) -> p kt l", kt=nkt)
    wq_r = wq.rearrange("p (kt g) -> p kt g", kt=nkt)
    wk_r = wk.rearrange("p (kt g) -> p kt g", kt=nkt)
    wv_r = wv.rearrange("p (kt g) -> p kt g", kt=KT)

    with tile.TileContext(nc) as tc, ExitStack() as ctx:
        persist = ctx.enter_context(tc.tile_pool(name="persist", bufs=1))
        spool = ctx.enter_context(tc.tile_pool(name="spool", bufs=2, space="PSUM"))
        opool = ctx.enter_context(tc.tile_pool(name="opool", bufs=2, space="PSUM"))
        ppool = ctx.enter_context(tc.tile_pool(name="ppool", bufs=2, space="PSUM"))
        ptpool = ctx.enter_context(tc.tile_pool(name="ptpool", bufs=4))
        accpool = ctx.enter_context(tc.tile_pool(name="accpool", bufs=2))
        rpool = ctx.enter_context(tc.tile_pool(name="rpool", bufs=2))
        bcpool = ctx.enter_context(tc.tile_pool(name="bcpool", bufs=2))
        popool = ctx.enter_context(tc.tile_pool(name="popool", bufs=2))

        # ---------------- persistent SBUF ----------------
        xT_sb = persist.tile([128, nkt, L], BF16, tag="xT")
        yT_sb = persist.tile([128, nkt, L], BF16, tag="yT")
        wq_sb = persist.tile([128, nkt, 256], BF16, tag="wq")
        wk_sb = persist.tile([128, nkt, 256], BF16, tag="wk")
        wv_sb = persist.tile([128, KT, GW], BF16, tag="wv")
        wpa_sb = persist.tile([128, D], BF16, tag="wpa")
        wpb_sb = persist.tile([64, D], BF16, tag="wpb")

        qT_p = persist.tile([128, L], BF16, tag="qTp")   # heads 0,1 stacked
        qT_2 = persist.tile([128, L], BF16, tag="qT2")   # head 2, dup halves
        kT_p = persist.tile([128, L], BF16, tag="kTp")
        kT_2 = persist.tile([128, L], BF16, tag="kT2")
        v_sb = persist.tile([128, NM, GW], BF16, tag="v")
        on_p = persist.tile([128, L], BF16, tag="onp")   # normalized O^T heads 0,1
        on_2 = persist.tile([64, L], BF16, tag="on2")    # head 2

        ones_col = persist.tile([128, 1], BF16, tag="onesc")
        scr = persist.tile([128, 64], BF16, tag="scr")
        nc.vector.memset(ones_col, 1.0)
        nc.vector.memset(scr, 1.0)

        # ---------------- input DMA, need-ordered ----------------
        # gates: k0h2 <- wk + yT[:512]; q2(0,1) <- wq + xT[:1024];
        # v(0..7) <- wv + yT[:1024]; later halves stream in behind.
        h1 = slice(L // 2, L)
        nc.sync.dma_start(out=wk_sb[:, :, 128:256], in_=wk_r[:, :, 128:256])
        for kt in range(nkt):
            nc.sync.dma_start(out=yT_sb[:, kt, 0:512], in_=yT_r[:, kt, 0:512])
        nc.sync.dma_start(out=wq_sb[:, :, 128:256], in_=wq_r[:, :, 128:256])
        for kt in range(nkt):
            nc.sync.dma_start(out=xT_sb[:, kt, 0:512], in_=xT_r[:, kt, 0:512])
        for kt in range(nkt):
            nc.sync.dma_start(out=xT_sb[:, kt, 512:1024], in_=xT_r[:, kt, 512:1024])
        nc.sync.dma_start(out=wv_sb, in_=wv_r)
        for kt in range(nkt):
            nc.sync.dma_start(out=yT_sb[:, kt, 512:1024], in_=yT_r[:, kt, 512:1024])
        nc.sync.dma_start(out=wk_sb[:, :, 0:128], in_=wk_r[:, :, 0:128])
        nc.sync.dma_start(out=wq_sb[:, :, 0:128], in_=wq_r[:, :, 0:128])
        for kt in range(nkt):
            nc.sync.dma_start(out=yT_sb[:, kt, h1], in_=yT_r[:, kt, h1])
        for kt in range(nkt):
            nc.sync.dma_start(out=xT_sb[:, kt, h1], in_=xT_r[:, kt, h1])
        nc.sync.dma_start(out=wpa_sb, in_=wpa)
        nc.sync.dma_start(out=wpb_sb, in_=wpb)

        def aux_ps():
            return ppool.tile([128, 512], F32, tag="aux", name="aux_ps")

        # ---------------- PE warm-up during the DMA wait ----------------
        wps = aux_ps()
        for _ in range(40):
            nc.tensor.matmul(wps[0:64, 0:64], scr, scr, start=True, stop=True)

        # ---------------- unit emitters ----------------
        def make_kq(j, half, w_sb, act_sb, dst):
            """K/Q projection for l/m-chunk j: two ~0.65us parts."""
            sl = slice(j * 512, (j + 1) * 512)
            cs = slice(128 * half, 128 * (half + 1))
            st = {}

            def part(p):
                if p == 0:
                    st["ps"] = aux_ps()
                ps = st["ps"]
                k0, k1 = (0, nkt // 2) if p == 0 else (nkt // 2, nkt)
                for kt in range(k0, k1):
                    nc.tensor.matmul(ps, w_sb[:, kt, cs], act_sb[:, kt, sl],
                                     start=(kt == 0), stop=(kt == nkt - 1))
                if p == 1:
                    nc.vector.tensor_copy(dst[:, sl], ps)
            return part

        def v_unit(m):
            ms = slice(m * 128, (m + 1) * 128)
            ps = aux_ps()
            for kt in range(KT):
                nc.tensor.matmul(ps[:, 0:GW], yT_sb[:, kt, ms], wv_sb[:, kt, :],
                                 start=(kt == 0), stop=(kt == KT - 1))
            nc.vector.tensor_copy(v_sb[:, m, :], ps[:, 0:GW])

        def p_unit(lc, o, tail=False):
            sl = slice(lc * 512, (lc + 1) * 512)
            osl = slice(o * 128, (o + 1) * 128)
            ps = aux_ps()
            if tail and WARM_TAIL:  # keep the PE HAM-warm through the norm window
                for _ in range(3):
                    nc.tensor.matmul(ps[0:64, 0:64], scr, scr, start=True,
                                     stop=True, skip_group_check=True)
            nc.tensor.matmul(ps, wpa_sb[:, osl], on_p[:, sl], start=True, stop=False)
            nc.tensor.matmul(ps, wpb_sb[:, osl], on_2[:, sl], start=False, stop=True)
            po = popool.tile([128, 512], F32, tag="po")
            if tail and o % 2 == 0 and SCALAR_PO:
                nc.scalar.copy(out=po, in_=ps)
            else:
                nc.vector.tensor_copy(po, ps)
            nc.sync.dma_start(out=pT[osl, sl], in_=po)

        # ---------------- attention iterations ----------------
        def iter_a(lc, m, o_ps, acc):
            sl = slice(lc * 512, (lc + 1) * 512)
            ms = slice(m * 128, (m + 1) * 128)
            s_ps = spool.tile([128, 1024], F32, tag="s")
            nc.tensor.matmul(s_ps[:, 0:512], kT_p[0:64, ms], qT_p[0:64, sl],
                             tile_position=(0, 0), start=True, stop=True)
            nc.tensor.matmul(s_ps[:, 512:1024], kT_p[64:128, ms], qT_p[64:128, sl],
                             tile_position=(64, 0), start=True, stop=True)
            pt = acc if m == 0 else ptpool.tile([128, 1024], BF16, tag="pt")
            nc.scalar.activation(pt, s_ps, EXP, scale=SCALE)
            nc.tensor.matmul(o_ps[0:64, :], v_sb[:, m, 0:64], pt[:, 0:512],
                             tile_position=(0, 0), start=(m == 0), stop=(m == NM - 1),
                             skip_group_check=True)
            nc.tensor.matmul(o_ps[64:128, :], v_sb[:, m, 64:128], pt[:, 512:1024],
                             tile_position=(0, 64), start=(m == 0), stop=(m == NM - 1),
                             skip_group_check=True)
            if m > 0:
                nc.vector.tensor_add(acc, acc, pt)

        def iter_b(pair, m, o_ps, acc):
            sl0 = slice(pair * 1024, pair * 1024 + 512)
            sl1 = slice(pair * 1024 + 512, (pair + 1) * 1024)
            ms = slice(m * 128, (m + 1) * 128)
            s_ps = spool.tile([128, 1024], F32, tag="s")
            nc.tensor.matmul(s_ps[:, 0:512], kT_2[0:64, ms], qT_2[0:64, sl0],
                             tile_position=(0, 0), start=True, stop=True)
            nc.tensor.matmul(s_ps[:, 512:1024], kT_2[64:128, ms], qT_2[64:128, sl1],
                             tile_position=(64, 0), start=True, stop=True)
            pt = acc if m == 0 else ptpool.tile([128, 1024], BF16, tag="pt")
            nc.scalar.activation(pt, s_ps, EXP, scale=SCALE)
            nc.tensor.matmul(o_ps[0:64, :], v_sb[:, m, 128:GW], pt[:, 0:512],
                             tile_position=(0, 0), start=(m == 0), stop=(m == NM - 1),
                             skip_group_check=True)
            nc.tensor.matmul(o_ps[64:128, :], v_sb[:, m, 128:GW], pt[:, 512:1024],
                             tile_position=(0, 64), start=(m == 0), stop=(m == NM - 1),
                             skip_group_check=True)
            if m > 0:
                nc.vector.tensor_add(acc, acc, pt)

        def run_group(kind, idx, extras, warm=0):
            o_ps = opool.tile([128, 512], F32, tag="o")
            acc = accpool.tile([128, 1024], BF16, tag="acc")
            for _ in range(warm if WARM_GROUP else 0):  # keep PE HAM-warm
                nc.tensor.matmul(o_ps[0:64, 0:64], scr, scr, start=True,
                                 stop=True, skip_group_check=True)
            for m in range(NM):
                for fn in extras.get(m, ()):
                    fn()
                if kind == "A":
                    iter_a(idx, m, o_ps, acc)
                else:
                    iter_b(idx, m, o_ps, acc)
            # softmax denominators + normalization
            den0 = aux_ps()
            nc.tensor.matmul(den0[0:64, :], scr, acc[:, 0:512], start=True, stop=True)
            den1 = aux_ps()
            nc.tensor.matmul(den1[0:64, :], scr, acc[:, 512:1024], start=True, stop=True)
            bc0 = bcpool.tile([64, 512], F32, tag="bc0")
            bc1 = bcpool.tile([64, 512], F32, tag="bc1")
            nc.vector.reciprocal_approx_fast(out=bc0, in_=den0[0:64, :])
            nc.vector.reciprocal_approx_fast(out=bc1, in_=den1[0:64, :])
            if kind == "A":
                sl = slice(idx * 512, (idx + 1) * 512)
                nc.vector.tensor_mul(on_p[0:64, sl], o_ps[0:64, :], bc0)
                nc.vector.tensor_mul(on_p[64:128, sl], o_ps[64:128, :], bc1)
            else:
                sl0 = slice(idx * 1024, idx * 1024 + 512)
                sl1 = slice(idx * 1024 + 512, (idx + 1) * 1024)
                nc.vector.tensor_mul(on_2[:, sl0], o_ps[0:64, :], bc0)
                nc.vector.tensor_mul(on_2[:, sl1], o_ps[64:128, :], bc1)

        # ---------------- prologue ----------------
        k0h2 = make_kq(0, 1, wk_sb, yT_sb, kT_2)
        k0h2(0); k0h2(1)
        q2_0 = make_kq(0, 1, wq_sb, xT_sb, qT_2)
        q2_0(0); q2_0(1)
        q2_1 = make_kq(1, 1, wq_sb, xT_sb, qT_2)
        q2_1(0); q2_1(1)
        v_unit(0)
        v_unit(1)

        # ---------------- group schedule ----------------
        k1h2 = make_kq(1, 1, wk_sb, yT_sb, kT_2)
        k2h2 = make_kq(2, 1, wk_sb, yT_sb, kT_2)
        k3h2 = make_kq(3, 1, wk_sb, yT_sb, kT_2)
        k0h01 = make_kq(0, 0, wk_sb, yT_sb, kT_p)
        run_group("B", 0, {
            0: [lambda: v_unit(2)],
            1: [lambda: v_unit(3)],
            2: [lambda: k1h2(0)],
            3: [lambda: k1h2(1), lambda: v_unit(4)],
            4: [lambda: v_unit(5)],
            5: [lambda: v_unit(6)],
            6: [lambda: k2h2(0), lambda: v_unit(7)],
            7: [lambda: k2h2(1), lambda: v_unit(8)],
            8: [lambda: v_unit(9)],
            9: [lambda: v_unit(10)],
            10: [lambda: k3h2(0), lambda: v_unit(11)],
            11: [lambda: k3h2(1), lambda: v_unit(12)],
            12: [lambda: v_unit(13)],
            13: [lambda: v_unit(14)],
            14: [lambda: v_unit(15), lambda: k0h01(0)],
            15: [lambda: k0h01(1)],
        })

        q0h01 = make_kq(0, 0, wq_sb, xT_sb, qT_p)
        k1h01 = make_kq(1, 0, wk_sb, yT_sb, kT_p)
        k2h01 = make_kq(2, 0, wk_sb, yT_sb, kT_p)
        k3h01 = make_kq(3, 0, wk_sb, yT_sb, kT_p)
        q2_2 = make_kq(2, 1, wq_sb, xT_sb, qT_2)
        q2_3 = make_kq(3, 1, wq_sb, xT_sb, qT_2)
        q0h01(0); q0h01(1)
        run_group("A", 0, {
            0: [lambda: k1h01(0)],
            1: [lambda: k1h01(1)],
            3: [lambda: q2_2(0)],
            4: [lambda: q2_2(1)],
            6: [lambda: k2h01(0)],
            7: [lambda: k2h01(1)],
            8: [lambda: q2_3(0)],
            9: [lambda: q2_3(1)],
            10: [lambda: k3h01(0)],
            11: [lambda: k3h01(1)],
        })

        q1h01 = make_kq(1, 0, wq_sb, xT_sb, qT_p)
        run_group("B", 1, {
            1: [lambda: p_unit(0, 0)],
            3: [lambda: p_unit(0, 1)],
            5: [lambda: p_unit(0, 2)],
            7: [lambda: p_unit(0, 3)],
            9: [lambda: p_unit(0, 4)],
            11: [lambda: p_unit(0, 5)],
            13: [lambda: q1h01(0)],
            14: [lambda: q1h01(1)],
        })

        q2h01 = make_kq(2, 0, wq_sb, xT_sb, qT_p)
        run_group("A", 1, {
            5: [lambda: q2h01(0)],
            6: [lambda: q2h01(1)],
        })

        q3h01 = make_kq(3, 0, wq_sb, xT_sb, qT_p)
        run_group("A", 2, {
            1: [lambda: p_unit(1, 0)],
            3: [lambda: p_unit(1, 1)],
            5: [lambda: p_unit(1, 2)],
            7: [lambda: p_unit(1, 3)],
            9: [lambda: p_unit(1, 4)],
            11: [lambda: p_unit(1, 5)],
            13: [lambda: q3h01(0)],
            14: [lambda: q3h01(1)],
        })

        run_group("A", 3, {
            1: [lambda: p_unit(2, 0)],
            3: [lambda: p_unit(2, 1)],
            5: [lambda: p_unit(2, 2)],
            7: [lambda: p_unit(2, 3)],
            9: [lambda: p_unit(2, 4)],
            11: [lambda: p_unit(2, 5)],
        })

        # tail: p_proj(3)
        for o in range(6):
            p_unit(3, o, tail=True)

    nc.finalize()
    return nc


def _pack_kt(t):
    """[da, W] -> [128, nkt*W] (kt-major blocks interleaved to partitions)."""
    da, w = t.shape
    nkt = da // 128
    return np.ascontiguousarray(
        t.reshape(nkt, 128, w).transpose(1, 0, 2).reshape(128, nkt * w)
    ).astype(bfloat16)


def _prep_act(a, aug):
    """[L, D] activations -> [128, nkt*L] (+ ones row when aug)."""
    da = (KTA if aug else KT) * 128
    t = np.zeros((da, L), dtype=np.float32)
    t[:D] = a.T
    if aug:
        t[D] = 1.0
    return _pack_kt(t)


def _prep_wqk(w_rows, b_rows, aug):
    """[GW, D] weight rows (+ [GW] bias) -> [128, nkt*256] packed lhsT.

    cols 0:128 = heads 0,1; cols 128:192 = head 2; cols 192:256 = head 2 dup.
    """
    da = (KTA if aug else KT) * 128
    t = np.zeros((da, 256), dtype=np.float32)
    t[:D, 0:128] = w_rows[0:128].T
    t[:D, 128:192] = w_rows[128:192].T
    t[:D, 192:256] = w_rows[128:192].T
    if aug:
        t[D, 0:128] = b_rows[0:128]
        t[D, 128:192] = b_rows[128:192]
        t[D, 192:256] = b_rows[128:192]
    return _pack_kt(t)


def _make_in_maps(x, y, Wq, bq, Wk, bk, Wv, bv, Wp, bp, aug):
    in_maps = []
    xTs = [_prep_act(np.asarray(x[b]), aug) for b in range(B)]
    yTs = [_prep_act(np.asarray(y[b]), aug) for b in range(B)]
    for core in range(8):
        b, g = divmod(core, 4)
        rows = slice(g * GW, (g + 1) * GW)
        wvT = np.zeros((KT * 128, GW), dtype=np.float32)
        wvT[:D] = Wv[rows].T
        in_maps.append({
            "xT": xTs[b],
            "yT": yTs[b],
            "wq": _prep_wqk(Wq[rows], bq[rows], aug),
            "wk": _prep_wqk(Wk[rows], bk[rows], aug),
            "wv": _pack_kt(wvT),
            "wpa": np.ascontiguousarray(Wp[:, rows].T[0:128]).astype(bfloat16),
            "wpb": np.ascontiguousarray(Wp[:, rows].T[128:GW]).astype(bfloat16),
        })
    return in_maps


def _combine(results, Wv, Wp, bp, bv):
    out = np.zeros((B, L, D), dtype=np.float32)
    for core in range(8):
        b = core // 4
        out[b] += results[core]["pT"].T
    out += (Wp @ bv + bp)[None, None, :]
    return out


_NC = {}


def _get_nc(aug):
    if aug not in _NC:
        _NC[aug] = _build_program(KTA if aug else KT)
    return _NC[aug]


def run(inputs, trace=False, trace_cores=None, **kwargs):
    aug = bool(np.any(inputs["bq"]) or np.any(inputs["bk"]))
    nc = _get_nc(aug)
    in_maps = _make_in_maps(aug=aug, **inputs)
    res = run_bass_kernel_spmd(
        nc, in_maps, core_ids=list(range(8)), trace=trace,
        trace_cores=trace_cores, **kwargs)
    out = _combine(res.results, inputs["Wv"], inputs["Wp"],
                   inputs["bp"], inputs["bv"])
    return out, res


def kernel(**inputs):
    inputs = {k: np.asarray(v) for k, v in inputs.items()}
    out, _ = run(inputs, trace=False)
    return out
